# revision 2
# baseline (speedup 1.0000x reference)
"""Trainium2 Bass kernel for nn_Model_22677427323544.

The circuit is AngleEmbedding(adds) followed by a batch-independent gate
sequence (all remaining gates depend only on params/weights/params2), then
<Z_0>. Algebraically:

    out[b] = psi0_b^H (U^H Z0 U) psi0_b          U = fixed 512x512 unitary
    psi0_b = D r_b,  D = diag((-i)^popcount(j)),  r_b real (Kronecker of
             [cos(t_i/2), sin(t_i/2)] per wire, wire 0 = MSB)
    =>  out[b] = r_b^T A r_b,   A = Re(D^H U^H Z0 U D)  real symmetric.

Host precomputes A (O(1) w.r.t. batch — pure parameter folding). The device
kernel, data-parallel over 8 cores (1024 samples each):
  1. sin/cos of adds/2 via ScalarE Sin (double-angle from t/4 for range safety)
  2. builds r as a 9-step Kronecker product on VectorE
  3. transposes r via TensorE to get the contraction layout
  4. Y = r @ A on TensorE (fp32), out = rowsum(r * Y) fused on VectorE
"""
import numpy as np

import concourse.bass as bass
import concourse.tile as tile
from concourse import bacc, mybir
from concourse import bass_utils
from concourse.masks import make_identity

N_WIRES = 9
DIM = 1 << N_WIRES            # 512
N_CORES = 8
B = 8192
B_LOC = B // N_CORES          # 1024
P = 128                       # partitions
G = B_LOC // P                # 8 batch groups per partition
KT = DIM // P                 # 4 contraction chunks
F32 = mybir.dt.float32

# ---------------------------------------------------------------------------
# Host-side parameter folding: A = Re(D^H U^H Z0 U D)
# ---------------------------------------------------------------------------

_X = np.array([[0, 1], [1, 0]], dtype=np.complex128)
_CNOT = np.array(
    [[1, 0, 0, 0], [0, 1, 0, 0], [0, 0, 0, 1], [0, 0, 1, 0]], dtype=np.complex128
)


def _rx(t):
    c, s = np.cos(t / 2), np.sin(t / 2)
    return np.array([[c, -1j * s], [-1j * s, c]])


def _ry(t):
    c, s = np.cos(t / 2), np.sin(t / 2)
    return np.array([[c, -s], [s, c]], dtype=np.complex128)


def _rz(t):
    return np.array([[np.exp(-0.5j * t), 0], [0, np.exp(0.5j * t)]])


def _rot(phi, theta, omega):
    return _rz(omega) @ _ry(theta) @ _rz(phi)


def _crz(t):
    return np.diag([1, 1, np.exp(-0.5j * t), np.exp(0.5j * t)]).astype(np.complex128)


def _crx(t):
    m = np.eye(4, dtype=np.complex128)
    m[2:, 2:] = _rx(t)
    return m


def _apply_1q(state, U, w):
    s = np.moveaxis(state, 1 + w, -1)
    s = np.einsum('ij,...j->...i', U, s)
    return np.moveaxis(s, -1, 1 + w)


def _apply_2q(state, U, c, t):
    s = np.moveaxis(state, (1 + c, 1 + t), (-2, -1))
    shp = s.shape
    s = s.reshape(shp[:-2] + (4,))
    s = np.einsum('ij,...j->...i', U, s)
    return np.moveaxis(s.reshape(shp), (-2, -1), (1 + c, 1 + t))


def _entangle_block(state, p):
    j = 0
    for i in range(N_WIRES):
        ip = (i + 1) % N_WIRES
        state = _apply_1q(state, _ry(p[j]), i)
        state = _apply_1q(state, _ry(p[j + 1]), ip)
        state = _apply_2q(state, _CNOT, i, ip)
        state = _apply_2q(state, _crz(p[j + 2]), i, ip)
        state = _apply_1q(state, _X, ip)
        state = _apply_2q(state, _crx(p[j + 3]), i, ip)
        j += 4
    return state


def _sel_layer(state, w, r):
    for i in range(N_WIRES):
        state = _apply_1q(state, _rot(w[i, 0], w[i, 1], w[i, 2]), i)
    for i in range(N_WIRES):
        state = _apply_2q(state, _CNOT, i, (i + r) % N_WIRES)
    return state


def _compute_A(params, weights, params2):
    params = np.asarray(params, np.float64)
    weights = np.asarray(weights, np.float64)
    params2 = np.asarray(params2, np.float64)
    state = np.eye(DIM, dtype=np.complex128).reshape((DIM,) + (2,) * N_WIRES)
    for l in range(3):
        state = _entangle_block(state, params[l * 36:(l + 1) * 36])
    for l in range(3):
        state = _sel_layer(state, weights[l], (l % (N_WIRES - 1)) + 1)
    for l in range(5):
        state = _entangle_block(state, params2[l * 36:(l + 1) * 36])
    U = state.reshape(DIM, DIM).T
    z = np.where(np.arange(DIM) < DIM // 2, 1.0, -1.0)
    M = U.conj().T @ (z[:, None] * U)
    pc = np.array([bin(j).count('1') for j in range(DIM)])
    d = (-1j) ** pc
    A = (np.conj(d)[:, None] * M * d[None, :]).real
    return np.ascontiguousarray(A, dtype=np.float32)


# ---------------------------------------------------------------------------
# Device program (per core: 1024 samples)
# ---------------------------------------------------------------------------

_PROGRAM = None


def _build_program():
    nc = bacc.Bacc("TRN2", target_bir_lowering=False, debug=False,
                   num_devices=N_CORES)
    adds_ext = nc.dram_tensor("adds", [B_LOC, N_WIRES], F32,
                              kind="ExternalInput").ap()
    amat_ext = nc.dram_tensor("amat", [DIM, DIM], F32,
                              kind="ExternalInput").ap()
    out_ext = nc.dram_tensor("out", [B_LOC], F32, kind="ExternalOutput").ap()

    with tile.TileContext(nc) as tc:
        with (
            tc.tile_pool(name="const", bufs=1) as cpool,
            tc.tile_pool(name="work", bufs=2) as wpool,
            tc.tile_pool(name="psum_t", bufs=2, space="PSUM") as pt,
            tc.tile_pool(name="psum_y", bufs=4, space="PSUM") as py,
        ):
            # A matrix: amat_sb[k_lo, k_hi, n] = A[k_hi*128 + k_lo, n]
            amat_sb = cpool.tile([P, KT, DIM], F32)
            a_view = amat_ext.rearrange("(kh kl) n -> kl kh n", kl=P)
            for kh in range(KT):
                nc.sync.dma_start(amat_sb[:, kh, :], a_view[:, kh, :])

            # adds shard: sample b = p*G + g  ->  adds_sb[p, g, :]
            adds_sb = cpool.tile([P, G, N_WIRES], F32)
            nc.sync.dma_start(adds_sb[:], adds_ext.rearrange("(p g) i -> p g i", g=G))

            ident = cpool.tile([P, P], F32)
            make_identity(nc, ident[:])
            halfpi = cpool.tile([P, 1], F32)
            nc.vector.memset(halfpi[:], float(np.pi / 2))

            # u = sin(t/4), v = cos(t/4); then c = 1-2u^2, s = 2uv
            u = cpool.tile([P, G, N_WIRES], F32)
            v = cpool.tile([P, G, N_WIRES], F32)
            nc.scalar.activation(u[:], adds_sb[:], mybir.ActivationFunctionType.Sin,
                                 scale=0.25)
            nc.scalar.activation(v[:], adds_sb[:], mybir.ActivationFunctionType.Sin,
                                 scale=-0.25, bias=halfpi[:])
            # cs[p, g, 0, i] = cos(t_i/2), cs[p, g, 1, i] = sin(t_i/2)
            cs = cpool.tile([P, G, 2, N_WIRES], F32)
            nc.vector.scalar_tensor_tensor(
                out=cs[:, :, 1, :], in0=u[:], scalar=2.0, in1=v[:],
                op0=mybir.AluOpType.mult, op1=mybir.AluOpType.mult)
            usq = cpool.tile([P, G, N_WIRES], F32)
            nc.vector.tensor_mul(usq[:], u[:], u[:])
            nc.vector.tensor_scalar(
                out=cs[:, :, 0, :], in0=usq[:], scalar1=-2.0, scalar2=1.0,
                op0=mybir.AluOpType.mult, op1=mybir.AluOpType.add)

            # Kronecker build of r: rmag[p, g, j] = prod_i f_i(bit_i(j))
            sA = cpool.tile([P, G, 128], F32)
            sB = cpool.tile([P, G, 256], F32)
            rmag = cpool.tile([P, G, DIM], F32)
            nc.vector.tensor_copy(sA[:, :, :2], cs[:, :, :, 0])
            cur = sA
            for w in range(1, N_WIRES):
                L = 1 << w
                nxt = rmag if w == N_WIRES - 1 else (sB if cur is sA else sA)
                out_ap = nxt[:, :, :2 * L].rearrange("p g (l b) -> p g l b", b=2)
                in0 = cur[:, :, :L, None].to_broadcast((P, G, L, 2))
                in1 = cs[:, :, :, w][:, :, None, :].to_broadcast((P, G, L, 2))
                nc.vector.tensor_mul(out_ap, in0, in1)
                cur = nxt

            # Transpose to contraction layout:
            # rmagT[j_lo, k, g*128 + p] = rmag[p, g, k*128 + j_lo]
            rmagT = cpool.tile([P, KT, B_LOC], F32)
            for g in range(G):
                tp = pt.tile([P, DIM], F32, tag="tp")
                for k in range(KT):
                    nc.tensor.transpose(tp[:, k * P:(k + 1) * P],
                                        rmag[:, g, k * P:(k + 1) * P], ident[:])
                nc.vector.tensor_copy(
                    rmagT[:, :, g * P:(g + 1) * P],
                    tp[:].rearrange("p (k x) -> p k x", k=KT))

            # Y_g = r_g @ A   (PSUM, fp32);  out[:, g] = rowsum(Y_g * r_g)
            res = cpool.tile([P, G], F32)
            for g in range(G):
                yp = py.tile([P, DIM], F32, tag="yp")
                for k in range(KT):
                    nc.tensor.matmul(yp[:], lhsT=rmagT[:, k, g * P:(g + 1) * P],
                                     rhs=amat_sb[:, k, :],
                                     start=(k == 0), stop=(k == KT - 1))
                wscr = wpool.tile([P, DIM], F32, tag="wscr")
                nc.vector.tensor_mul(wscr[:], yp[:], rmag[:, g, :])
                nc.vector.tensor_reduce(res[:, g:g + 1], wscr[:],
                                        mybir.AxisListType.X, mybir.AluOpType.add)

            nc.sync.dma_start(out_ext.rearrange("(p g) -> p g", g=G), res[:])

    nc.compile()
    return nc


def _get_program():
    global _PROGRAM
    if _PROGRAM is None:
        _PROGRAM = _build_program()
    return _PROGRAM


def kernel(adds, params, weights, params2):
    adds = np.ascontiguousarray(np.asarray(adds), dtype=np.float32)
    A = _compute_A(params, weights, params2)
    nc = _get_program()
    in_maps = [
        {"adds": adds[i * B_LOC:(i + 1) * B_LOC], "amat": A}
        for i in range(N_CORES)
    ]
    results = bass_utils.run_bass_kernel_spmd(nc, in_maps, list(range(N_CORES))).results
    return np.concatenate([results[i]["out"] for i in range(N_CORES)])


# revision 3
# speedup vs baseline: 1.7082x; 1.7082x over previous
"""Trainium2 Bass kernel for nn_Model_22677427323544.

The circuit is AngleEmbedding(adds) followed by a batch-independent gate
sequence (all remaining gates depend only on params/weights/params2), then
<Z_0>. Algebraically:

    out[b] = psi0_b^H (U^H Z0 U) psi0_b          U = fixed 512x512 unitary
    psi0_b = D r_b,  D = diag((-i)^popcount(j)),  r_b real (Kronecker of
             [cos(t_i/2), sin(t_i/2)] per wire, wire 0 = MSB)
    =>  out[b] = r_b^T A r_b,   A = Re(D^H U^H Z0 U D)  real symmetric.

Host precomputes A (O(1) w.r.t. batch — pure parameter folding). The device
kernel, data-parallel over 8 cores (1024 samples each):
  1. sin/cos of adds/2 via ScalarE Sin (double-angle from t/4 for range safety)
  2. builds r as a 9-step Kronecker product on VectorE (wires processed
     8..0 so each step appends at the MSB => contiguous inner runs)
  3. transposes r via TensorE; PSUM->SBUF copies on ScalarE cast to fp32r
  4. Y = r @ A on TensorE in fp32r (1 col/cycle, ~17-bit mantissa)
  5. out = rowsum(Y * r) fused in one VectorE scalar_tensor_tensor per group
"""
import numpy as np
import ml_dtypes

import concourse.bass as bass
import concourse.tile as tile
from concourse import bacc, mybir
from concourse import bass_utils

N_WIRES = 9
DIM = 1 << N_WIRES            # 512
N_CORES = 8
B = 8192
B_LOC = B // N_CORES          # 1024
P = 128                       # partitions
G = B_LOC // P                # 8 batch groups per partition
KT = DIM // P                 # 4 contraction chunks
F32 = mybir.dt.float32
F32R = mybir.dt.float32r

# ---------------------------------------------------------------------------
# Host-side parameter folding: A = Re(D^H U^H Z0 U D)
# ---------------------------------------------------------------------------

_X = np.array([[0, 1], [1, 0]], dtype=np.complex128)
_CNOT = np.array(
    [[1, 0, 0, 0], [0, 1, 0, 0], [0, 0, 0, 1], [0, 0, 1, 0]], dtype=np.complex128
)


def _rx(t):
    c, s = np.cos(t / 2), np.sin(t / 2)
    return np.array([[c, -1j * s], [-1j * s, c]])


def _ry(t):
    c, s = np.cos(t / 2), np.sin(t / 2)
    return np.array([[c, -s], [s, c]], dtype=np.complex128)


def _rz(t):
    return np.array([[np.exp(-0.5j * t), 0], [0, np.exp(0.5j * t)]])


def _rot(phi, theta, omega):
    return _rz(omega) @ _ry(theta) @ _rz(phi)


def _crz(t):
    return np.diag([1, 1, np.exp(-0.5j * t), np.exp(0.5j * t)]).astype(np.complex128)


def _crx(t):
    m = np.eye(4, dtype=np.complex128)
    m[2:, 2:] = _rx(t)
    return m


def _apply_1q(state, U, w):
    s = np.moveaxis(state, 1 + w, -1)
    s = np.einsum('ij,...j->...i', U, s)
    return np.moveaxis(s, -1, 1 + w)


def _apply_2q(state, U, c, t):
    s = np.moveaxis(state, (1 + c, 1 + t), (-2, -1))
    shp = s.shape
    s = s.reshape(shp[:-2] + (4,))
    s = np.einsum('ij,...j->...i', U, s)
    return np.moveaxis(s.reshape(shp), (-2, -1), (1 + c, 1 + t))


def _entangle_block(state, p):
    j = 0
    for i in range(N_WIRES):
        ip = (i + 1) % N_WIRES
        state = _apply_1q(state, _ry(p[j]), i)
        state = _apply_1q(state, _ry(p[j + 1]), ip)
        state = _apply_2q(state, _CNOT, i, ip)
        state = _apply_2q(state, _crz(p[j + 2]), i, ip)
        state = _apply_1q(state, _X, ip)
        state = _apply_2q(state, _crx(p[j + 3]), i, ip)
        j += 4
    return state


def _sel_layer(state, w, r):
    for i in range(N_WIRES):
        state = _apply_1q(state, _rot(w[i, 0], w[i, 1], w[i, 2]), i)
    for i in range(N_WIRES):
        state = _apply_2q(state, _CNOT, i, (i + r) % N_WIRES)
    return state


def _round_fp32r(x):
    """Round fp32 to the 2xbf16-decomposable subset (fp32r)."""
    hi = x.astype(ml_dtypes.bfloat16).astype(np.float32)
    lo = (x - hi).astype(ml_dtypes.bfloat16).astype(np.float32)
    return hi + lo


def _compute_A(params, weights, params2):
    params = np.asarray(params, np.float64)
    weights = np.asarray(weights, np.float64)
    params2 = np.asarray(params2, np.float64)
    state = np.eye(DIM, dtype=np.complex128).reshape((DIM,) + (2,) * N_WIRES)
    for l in range(3):
        state = _entangle_block(state, params[l * 36:(l + 1) * 36])
    for l in range(3):
        state = _sel_layer(state, weights[l], (l % (N_WIRES - 1)) + 1)
    for l in range(5):
        state = _entangle_block(state, params2[l * 36:(l + 1) * 36])
    U = state.reshape(DIM, DIM).T
    z = np.where(np.arange(DIM) < DIM // 2, 1.0, -1.0)
    M = U.conj().T @ (z[:, None] * U)
    pc = np.array([bin(j).count('1') for j in range(DIM)])
    d = (-1j) ** pc
    A = (np.conj(d)[:, None] * M * d[None, :]).real
    return _round_fp32r(np.ascontiguousarray(A, dtype=np.float32))


# ---------------------------------------------------------------------------
# Device program (per core: 1024 samples; sample index = p*G + g)
# ---------------------------------------------------------------------------

_PROGRAM = None


def _build_program():
    nc = bacc.Bacc("TRN2", target_bir_lowering=False, debug=False,
                   num_devices=N_CORES)
    adds_ext = nc.dram_tensor("adds", [B_LOC, N_WIRES], F32,
                              kind="ExternalInput").ap()
    amat_ext = nc.dram_tensor("amat", [DIM, DIM], F32R,
                              kind="ExternalInput").ap()
    out_ext = nc.dram_tensor("out", [B_LOC], F32, kind="ExternalOutput").ap()

    with tile.TileContext(nc) as tc:
        with (
            tc.tile_pool(name="const", bufs=1) as cpool,
            tc.tile_pool(name="work", bufs=2) as wpool,
            tc.tile_pool(name="psum_t", bufs=2, space="PSUM") as pt,
            tc.tile_pool(name="psum_y", bufs=4, space="PSUM") as py,
        ):
            # adds shard first (small, unblocks the whole front end)
            adds_sb = cpool.tile([P, G, N_WIRES], F32)
            nc.sync.dma_start(adds_sb[:], adds_ext.rearrange("(p g) i -> p g i", g=G))

            # A matrix (fp32r, host-rounded): amat_sb[k_lo, k_hi, n]
            amat_sb = cpool.tile([P, KT, DIM], F32R)
            a_view = amat_ext.rearrange("(kh kl) n -> kl kh n", kl=P)
            for kh in range(KT):
                nc.sync.dma_start(amat_sb[:, kh, :], a_view[:, kh, :])

            # identity for PE transpose
            ident = cpool.tile([P, P], F32)
            nc.gpsimd.memset(ident[:], 0.0)
            nc.gpsimd.affine_select(
                out=ident[:], in_=ident[:],
                compare_op=mybir.AluOpType.not_equal, fill=1.0,
                base=0, pattern=[[-1, P]], channel_multiplier=1)
            halfpi = cpool.tile([P, 1], F32)
            nc.vector.memset(halfpi[:], float(np.pi / 2))

            # u = sin(t/4), v = cos(t/4); c = 1-2u^2, s = 2uv
            u = cpool.tile([P, G, N_WIRES], F32)
            v = cpool.tile([P, G, N_WIRES], F32)
            nc.scalar.activation(u[:], adds_sb[:], mybir.ActivationFunctionType.Sin,
                                 scale=0.25)
            nc.scalar.activation(v[:], adds_sb[:], mybir.ActivationFunctionType.Sin,
                                 scale=-0.25, bias=halfpi[:])
            # cs[p, g, 0, i] = cos(t_i/2), cs[p, g, 1, i] = sin(t_i/2)
            cs = cpool.tile([P, G, 2, N_WIRES], F32)
            nc.vector.scalar_tensor_tensor(
                out=cs[:, :, 1, :], in0=u[:], scalar=2.0, in1=v[:],
                op0=mybir.AluOpType.mult, op1=mybir.AluOpType.mult)
            usq = cpool.tile([P, G, N_WIRES], F32)
            nc.vector.tensor_mul(usq[:], u[:], u[:])
            nc.vector.tensor_scalar(
                out=cs[:, :, 0, :], in0=usq[:], scalar1=-2.0, scalar2=1.0,
                op0=mybir.AluOpType.mult, op1=mybir.AluOpType.add)

            # Kronecker build of r, appending each new wire at the MSB:
            # process wires 8,7,...,0 so wire 0 ends up as the MSB (stride 256)
            # and wire 8 as the LSB — the reference flattening order.
            # step: out[p, g, b*L + m] = in[p, g, m] * cs[p, g, b, w]
            sA = cpool.tile([P, G, 128], F32)
            sB = cpool.tile([P, G, 256], F32)
            rmag = cpool.tile([P, G, DIM], F32)
            nc.vector.tensor_copy(sA[:, :, :2], cs[:, :, :, N_WIRES - 1])
            cur = sA
            for step in range(1, N_WIRES - 1):
                w = N_WIRES - 1 - step
                L = 1 << step
                nxt = sB if cur is sA else sA
                out_ap = nxt[:, :, :2 * L].rearrange("p g (b m) -> p g b m", b=2)
                in0 = cur[:, :, None, :L].to_broadcast((P, G, 2, L))
                in1 = cs[:, :, :, w][:, :, :, None].to_broadcast((P, G, 2, L))
                nc.vector.tensor_mul(out_ap, in0, in1)
                cur = nxt
            # last step (wire 0) split per group so downstream work pipelines
            HALF = DIM // 2
            for g in range(G):
                out_ap = rmag[:, g, :].rearrange("p (b m) -> p b m", b=2)
                in0 = cur[:, g, None, :].to_broadcast((P, 2, HALF))
                in1 = cs[:, g, :, 0][:, :, None].to_broadcast((P, 2, HALF))
                nc.vector.tensor_mul(out_ap, in0, in1)

            # Transpose to contraction layout (fp32 PE transpose), PSUM->SBUF
            # copy on ScalarE with cast to fp32r:
            # rmagT[j_lo, k, g*128 + p] = rmag[p, g, k*128 + j_lo]
            rmagT = cpool.tile([P, KT, B_LOC], F32R)
            res = cpool.tile([P, G], F32)
            for g in range(G):
                tp = pt.tile([P, DIM], F32, tag="tp")
                for k in range(KT):
                    nc.tensor.transpose(tp[:, k * P:(k + 1) * P],
                                        rmag[:, g, k * P:(k + 1) * P], ident[:])
                nc.scalar.copy(
                    rmagT[:, :, g * P:(g + 1) * P],
                    tp[:].rearrange("p (k x) -> p k x", k=KT))

                # Y_g = r_g @ A  (fp32r matmul, fp32 PSUM accumulate)
                yp = py.tile([P, DIM], F32, tag="yp")
                for k in range(KT):
                    nc.tensor.matmul(yp[:], lhsT=rmagT[:, k, g * P:(g + 1) * P],
                                     rhs=amat_sb[:, k, :],
                                     start=(k == 0), stop=(k == KT - 1))
                # out[:, g] = rowsum(Y_g * r_g), fused
                wscr = wpool.tile([P, DIM], F32, tag="wscr")
                nc.vector.scalar_tensor_tensor(
                    out=wscr[:], in0=yp[:], scalar=0.0, in1=rmag[:, g, :],
                    op0=mybir.AluOpType.add, op1=mybir.AluOpType.mult,
                    accum_out=res[:, g:g + 1])

            nc.sync.dma_start(out_ext.rearrange("(p g) -> p g", g=G), res[:])

    nc.compile()
    return nc


def _get_program():
    global _PROGRAM
    if _PROGRAM is None:
        _PROGRAM = _build_program()
    return _PROGRAM


def kernel(adds, params, weights, params2):
    adds = np.ascontiguousarray(np.asarray(adds), dtype=np.float32)
    A = _compute_A(params, weights, params2)
    nc = _get_program()
    in_maps = [
        {"adds": adds[i * B_LOC:(i + 1) * B_LOC], "amat": A}
        for i in range(N_CORES)
    ]
    results = bass_utils.run_bass_kernel_spmd(nc, in_maps, list(range(N_CORES))).results
    return np.concatenate([results[i]["out"] for i in range(N_CORES)])
